# revision 1
# baseline (speedup 1.0000x reference)
"""Trainium2 Bass kernel for nn_Block_60017872995079 (moe_routing).

Block: x = x + attn(LN1(x)) ; x = x + moe(LN2(x))
  attn: LN1 -> causal depthwise conv1d(k=3) -> qkv -> RoPE -> causal MHA -> proj
  moe: LN2 -> router argmax (4 experts) -> per-token expert GLU-MLP (SwiGLU)

Sharding over 8 NeuronCores (single SPMD program, per-core behavior driven
entirely by per-core input DATA so every core runs identical code):
  - Attention: token-parallel. Core c owns tokens [512c, 512c+512) (batch c//2).
    Host ships per-core x_q (own tokens + 128 ctx rows), x_kv (full batch,
    zero ctx rows) and per-core causal 0/1 masks.
  - MoE: expert-parallel. Core c owns expert c//2 and the tokens routed to it
    from global half c%2. Routing is computed on device: argmax via
    max/compare masks, stream compaction via sparse_gather, token dispatch via
    dma_gather (capacity 640 tokens/core, expected ~512, -1-padded tail).
  - LN2 output + router logits are AllGathered (one collective) so each core
    can gather any token's h2. MoE outputs return compact (640 rows + token
    ids); host does the final unpermute-add (the unshard step).

All matmuls run in float32r (fp32 bit layout, reduced-precision PE mode, full
speed at free-dim>=256; ~1.4e-4 rel err per matmul).
"""

import sys
import numpy as np
import ml_dtypes

sys.path.insert(0, "/opt/trn_rl_repo")

import concourse.bass as bass  # noqa: E402,F401
import concourse.tile as tile  # noqa: E402
import concourse.mybir as mybir  # noqa: E402
from concourse import bacc  # noqa: E402
from concourse.bass_utils import run_bass_kernel_spmd  # noqa: E402

f32 = mybir.dt.float32
f32r = mybir.dt.float32r
bf16 = mybir.dt.bfloat16
i16 = mybir.dt.int16
i32 = mybir.dt.int32
u32 = mybir.dt.uint32
Alu = mybir.AluOpType
Act = mybir.ActivationFunctionType

P = 128
B, T, C, NH, D, E, Dff = 4, 1024, 1024, 16, 4, 4, 4096
D = 64
TOK = B * T            # 4096
S = TOK // 8           # 512 tokens per core
CAP2 = 640             # capacity slots per core (half expert)
FW = TOK // 16         # wrapped free dim for routing (256)
CCW = 1152             # allgather width (bf16): 1152*2B = 9*256
EPS = 1e-5

_CACHE = {}


def _build_nc():
    nc = bacc.Bacc(None)

    x_q = nc.dram_tensor("x_q", [640, C], f32, kind="ExternalInput")
    x_kv = nc.dram_tensor("x_kv", [1152, C], f32, kind="ExternalInput")
    attn_w = nc.dram_tensor("attn_w", [C, 3 * C], f32r, kind="ExternalInput")
    proj_w = nc.dram_tensor("proj_w", [C, C], f32r, kind="ExternalInput")
    conv_w3 = nc.dram_tensor("conv_w3", [P, 8, 3], f32, kind="ExternalInput")
    ln1_w2 = nc.dram_tensor("ln1_w2", [P, 8], f32, kind="ExternalInput")
    ln1_b2 = nc.dram_tensor("ln1_b2", [P, 8], f32, kind="ExternalInput")
    ln2_w_rep = nc.dram_tensor("ln2_w_rep", [P, C], f32, kind="ExternalInput")
    ln2_b_rep = nc.dram_tensor("ln2_b_rep", [P, C], f32, kind="ExternalInput")
    cos_cm = nc.dram_tensor("cos_cm", [P, T + S], bf16, kind="ExternalInput")
    sin_cm = nc.dram_tensor("sin_cm", [P, T + S], bf16, kind="ExternalInput")
    rw = nc.dram_tensor("rw", [C, E], f32r, kind="ExternalInput")
    masks = nc.dram_tensor("masks", [8, P, S], bf16, kind="ExternalInput")
    wv_e = nc.dram_tensor("wv_e", [C, 2 * Dff], bf16, kind="ExternalInput")
    proj_e = nc.dram_tensor("proj_e", [Dff, C], bf16, kind="ExternalInput")
    onehot16 = nc.dram_tensor("onehot16", [16, E], f32, kind="ExternalInput")
    bounds16 = nc.dram_tensor("bounds16", [16, 2], f32, kind="ExternalInput")
    identity = nc.dram_tensor("identity", [P, P], f32r, kind="ExternalInput")
    ones16 = nc.dram_tensor("ones16", [P, 16], f32r, kind="ExternalInput")
    ones_row = nc.dram_tensor("ones_row", [1, P], f32r, kind="ExternalInput")
    ones_big = nc.dram_tensor("ones_big", [P, P], f32r, kind="ExternalInput")

    o_xmid = nc.dram_tensor("o_xmid", [S, C], f32, kind="ExternalOutput")
    o_moe = nc.dram_tensor("o_moe", [CAP2, C], f32, kind="ExternalOutput")
    o_idx = nc.dram_tensor("o_idx", [16, CAP2 // 16], f32, kind="ExternalOutput")
    o_cnt = nc.dram_tensor("o_cnt", [1, 1], f32, kind="ExternalOutput")

    cc_in = nc.dram_tensor("cc_in", [S, CCW], bf16)
    idx_dram = nc.dram_tensor("idx_dram", [16, CAP2 // 16], i16)
    cc_out = nc.dram_tensor("cc_out", [TOK, CCW], bf16, addr_space="Shared")


    with tile.TileContext(nc) as tc, nc.allow_low_precision(reason="f32r pipeline"):
        consts_cm = tc.tile_pool(name="consts", bufs=1)
        consts = consts_cm.__enter__()
        ident = consts.tile([P, P], f32r)
        nc.sync.dma_start(ident[:], identity[:, :])
        one16 = consts.tile([P, 16], f32r)
        nc.sync.dma_start(one16[:], ones16[:, :])
        onr = consts.tile([1, P], f32r)
        nc.sync.dma_start(onr[:], ones_row[:, :])
        onesb = consts.tile([P, P], f32r)
        nc.sync.dma_start(onesb[:], ones_big[:, :])
        cw3 = consts.tile([P, 8, 3], f32)
        nc.sync.dma_start(cw3[:], conv_w3[:, :, :])
        l1w = consts.tile([P, 8], f32)
        nc.sync.dma_start(l1w[:], ln1_w2[:, :])
        l1b = consts.tile([P, 8], f32)
        nc.sync.dma_start(l1b[:], ln1_b2[:, :])
        l2w = consts.tile([P, C], f32)
        nc.sync.dma_start(l2w[:], ln2_w_rep[:, :])
        l2b = consts.tile([P, C], f32)
        nc.sync.dma_start(l2b[:], ln2_b_rep[:, :])
        epst = consts.tile([P, 1], f32)
        nc.vector.memset(epst[:], EPS)

        BN_F = nc.vector.BN_STATS_FMAX  # 512
        n_sub = C // BN_F

        def ln_tile(pool, x_sb):
            stats = pool.tile([P, n_sub, nc.vector.BN_STATS_DIM], f32, tag="ln_stats")
            for sgi in range(n_sub):
                nc.vector.bn_stats(stats[:, sgi, :],
                                   x_sb[:, sgi * BN_F:(sgi + 1) * BN_F])
            mv = pool.tile([P, nc.vector.BN_AGGR_DIM], f32, tag="ln_mv")
            nc.vector.bn_aggr(mv[:], stats[:])
            sd = pool.tile([P, 1], f32, tag="ln_sd")
            nc.scalar.activation(sd[:], mv[:, 1:2], Act.Sqrt, bias=epst[:], scale=1.0)
            nc.vector.reciprocal(sd[:], sd[:])
            xn = pool.tile([P, C], f32r, tag="ln_xn")
            nc.vector.tensor_scalar(xn[:], x_sb[:], scalar1=mv[:, 0:1], scalar2=sd[:],
                                    op0=Alu.subtract, op1=Alu.mult)
            return xn

        def transpose_into(psum_pool, src_tile, dst, ti):
            """PE-transpose one [P, C] f32r token-tile into dst[:, ct, ti*128..]."""
            for cg in range(2):
                pst = psum_pool.tile([P, 512], f32r, tag="tr_ps")
                for cj in range(4):
                    ct = cg * 4 + cj
                    nc.tensor.transpose(pst[:, cj * P:(cj + 1) * P],
                                        src_tile[:, ct * P:(ct + 1) * P], ident[:])
                nc.vector.tensor_copy(
                    dst[:, cg * 4:(cg + 1) * 4, ti * P:(ti + 1) * P],
                    pst[:].rearrange("p (a b) -> p a b", b=P))

        def rope_cm(pool, ps_in, out_sb, out_sl, tcol0, w):
            shuf = pool.tile([P, w], f32, tag="rp_sh")
            smask = [8, 9, 10, 11, 12, 13, 14, 15, 0, 1, 2, 3, 4, 5, 6, 7,
                     24, 25, 26, 27, 28, 29, 30, 31, 16, 17, 18, 19, 20, 21, 22, 23]
            nc.vector.stream_shuffle(shuf[:], ps_in[:], smask)
            t1 = pool.tile([P, w], f32, tag="rp_t1")
            nc.vector.tensor_tensor(t1[:], ps_in[:], cosc[:, tcol0:tcol0 + w], Alu.mult)
            nc.vector.tensor_tensor(shuf[:], shuf[:], sinc[:, tcol0:tcol0 + w], Alu.mult)
            nc.vector.tensor_tensor(out_sb[out_sl], t1[:], shuf[:], Alu.add)

        # pool stack (LIFO): consts > bigD > bigB > [phase pools]
        bigD_cm = tc.tile_pool(name="bigD", bufs=1)
        bigD = bigD_cm.__enter__()
        bigB_cm = tc.tile_pool(name="bigB", bufs=1)
        bigB = bigB_cm.__enter__()

        # ============ PHASE A: LN1 + transpose + conv ============
        bigA_cm = tc.tile_pool(name="bigA", bufs=1)
        bigA = bigA_cm.__enter__()
        pA_cm = tc.tile_pool(name="pA", bufs=3)
        pA = pA_cm.__enter__()
        psA_cm = tc.tile_pool(name="psA", bufs=2, space="PSUM")
        psA = psA_cm.__enter__()

        h1t_kv = bigA.tile([P, 8, 1152], f32r)
        h1t_q = bigA.tile([P, 8, 640], f32r)
        for i in range(9):
            xt = pA.tile([P, C], f32, tag="xkv_in")
            nc.sync.dma_start(xt[:], x_kv[i * P:(i + 1) * P, :])
            transpose_into(psA, ln_tile(pA, xt), h1t_kv, i)
        for i in range(5):
            xt = pA.tile([P, C], f32, tag="xkv_in")
            nc.sync.dma_start(xt[:], x_q[i * P:(i + 1) * P, :])
            transpose_into(psA, ln_tile(pA, xt), h1t_q, i)

        hct_kv = bigB.tile([P, 8, T], f32r)
        hct_q = bigB.tile([P, 8, S], f32r)
        for ct in range(8):
            nc.vector.tensor_scalar(h1t_kv[:, ct, :], h1t_kv[:, ct, :],
                                    scalar1=l1w[:, ct:ct + 1], scalar2=l1b[:, ct:ct + 1],
                                    op0=Alu.mult, op1=Alu.add)
            nc.vector.tensor_scalar(h1t_q[:, ct, :], h1t_q[:, ct, :],
                                    scalar1=l1w[:, ct:ct + 1], scalar2=l1b[:, ct:ct + 1],
                                    op0=Alu.mult, op1=Alu.add)
        for src, dst, n in ((h1t_kv, hct_kv, T), (h1t_q, hct_q, S)):
            for ct in range(8):
                nc.gpsimd.tensor_scalar(dst[:, ct, :], src[:, ct, 126:126 + n],
                                        scalar1=cw3[:, ct, 0:1], scalar2=None,
                                        op0=Alu.mult)
                nc.vector.scalar_tensor_tensor(dst[:, ct, :], src[:, ct, 127:127 + n],
                                               cw3[:, ct, 1:2], dst[:, ct, :],
                                               op0=Alu.mult, op1=Alu.add)
                nc.vector.scalar_tensor_tensor(dst[:, ct, :], src[:, ct, 128:128 + n],
                                               cw3[:, ct, 2:3], dst[:, ct, :],
                                               op0=Alu.mult, op1=Alu.add)
        psA_cm.__exit__(None, None, None)
        pA_cm.__exit__(None, None, None)
        bigA_cm.__exit__(None, None, None)

        # ============ PHASE B+C: qkv + rope + attention, fused per head-pair ==
        pV_cm = tc.tile_pool(name="pV", bufs=1)
        pV = pV_cm.__enter__()
        pB_cm = tc.tile_pool(name="pB", bufs=2)
        pB = pB_cm.__enter__()
        pBs_cm = tc.tile_pool(name="pBs", bufs=2)
        pBs = pBs_cm.__enter__()
        psB_cm = tc.tile_pool(name="psB", bufs=2, space="PSUM")
        psB = psB_cm.__enter__()

        cosc = bigD.tile([P, T + S], bf16)
        nc.sync.dma_start(cosc[:], cos_cm[:, :])
        sinc = bigD.tile([P, T + S], bf16)
        nc.sync.dma_start(sinc[:], sin_cm[:, :])
        msk = bigD.tile([P, 8, S], bf16)
        nc.sync.dma_start(msk[:], masks[:, :, :].rearrange("k p q -> p k q"))
        oT = bigD.tile([P, 8, S], f32r)
        vt_all = pV.tile([P, 8, 16, 65], f32r)
        for ch in range(2):
            wtv = pV.tile([P, 8, 512], f32r, tag="aw_v5")
            nc.sync.dma_start(
                wtv[:], attn_w[:, 2 * C + ch * 512:2 * C + (ch + 1) * 512].rearrange(
                    "(ko ki) m -> ki ko m", ki=P))
            for tt in range(8):
                ps = psB.tile([P, 512], f32, tag="qkv_ps")
                for kt_i in range(8):
                    nc.tensor.matmul(ps[:], hct_kv[:, kt_i, tt * P:(tt + 1) * P],
                                     wtv[:, kt_i, :],
                                     start=(kt_i == 0), stop=(kt_i == 7))
                nc.scalar.copy(vt_all[:, tt, ch * 8:(ch + 1) * 8, 0:64],
                               ps[:].rearrange("p (h d) -> p h d", d=64))
        nc.sync.dma_start(
            vt_all[:, :, :, 64:65],
            onesb[:, 0:128].rearrange("p (a b) -> p a b", b=16)[:, :, :, None])

        for ct in range(8):
            # --- K channel-major for this chan tile (2 heads) ---
            kTc = pB.tile([P, T], f32r, tag="kTc")
            wt = pBs.tile([P, 8, P], f32r, tag="aw")
            nc.sync.dma_start(wt[:], attn_w[:, C + ct * P:C + (ct + 1) * P].rearrange(
                "(ko ki) m -> ki ko m", ki=P))
            for ch in range(2):
                ps = psB.tile([P, 512], f32, tag="qkv_ps")
                for kt_i in range(8):
                    nc.tensor.matmul(ps[:], wt[:, kt_i, :],
                                     hct_kv[:, kt_i, ch * 512:(ch + 1) * 512],
                                     start=(kt_i == 0), stop=(kt_i == 7))
                rope_cm(pBs, ps, kTc, (slice(None), slice(ch * 512, (ch + 1) * 512)),
                        ch * 512, 512)
            # --- Q ---
            qTc = pB.tile([P, S], f32r, tag="qTc")
            wt = pBs.tile([P, 8, P], f32r, tag="aw")
            nc.sync.dma_start(wt[:], attn_w[:, ct * P:(ct + 1) * P].rearrange(
                "(ko ki) m -> ki ko m", ki=P))
            ps = psB.tile([P, 512], f32, tag="qkv_ps")
            for kt_i in range(8):
                nc.tensor.matmul(ps[:], wt[:, kt_i, :], hct_q[:, kt_i, :],
                                 start=(kt_i == 0), stop=(kt_i == 7))
            rope_cm(pBs, ps, qTc, (slice(None), slice(0, S)), T, S)
            # --- attention for heads 2ct, 2ct+1 ---
            for hh in range(2):
                po = 64 * hh
                ps_o = psB.tile([65, S], f32, tag="pv_ps")
                for kt_i in range(8):
                    ps_s = psB.tile([P, S], f32, tag="s_ps")
                    nc.tensor.matmul(ps_s[:],
                                     kTc[po:po + 64, kt_i * P:(kt_i + 1) * P],
                                     qTc[po:po + 64, :], start=True, stop=True)
                    pt = pBs.tile([P, S], f32, tag="exp_t")
                    nc.scalar.activation(pt[:], ps_s[:], Act.Exp, scale=0.125)
                    ptm = pBs.tile([P, S], f32r, tag="ptm_t")
                    meng = nc.vector if kt_i % 2 == 0 else nc.gpsimd
                    meng.tensor_tensor(ptm[:], pt[:], msk[:, kt_i, :], Alu.mult)
                    nc.tensor.matmul(ps_o[:], vt_all[:, kt_i, 2 * ct + hh, :], ptm[:],
                                     start=(kt_i == 0), stop=(kt_i == 7))
                rc = pBs.tile([1, S], f32r, tag="rc_t")
                nc.vector.reciprocal(rc[:], ps_o[64:65, :])
                ps_b = psB.tile([64, S], f32, tag="bc_ps")
                nc.tensor.matmul(ps_b[:], onr[:, 0:64], rc[:], start=True, stop=True)
                bc = pBs.tile([64, S], f32, tag="bc_t")
                nc.scalar.copy(bc[:], ps_b[:])
                nc.vector.tensor_tensor(oT[po:po + 64, ct, :], ps_o[0:64, :], bc[:],
                                        Alu.mult)
        psB_cm.__exit__(None, None, None)
        pBs_cm.__exit__(None, None, None)
        pB_cm.__exit__(None, None, None)
        pV_cm.__exit__(None, None, None)
        bigB_cm.__exit__(None, None, None)

        # ============ PHASE D: proj + residual + LN2 + logits ============
        pDx_cm = tc.tile_pool(name="pDx", bufs=1)
        pDx = pDx_cm.__enter__()
        pD_cm = tc.tile_pool(name="pD", bufs=3)
        pD = pD_cm.__enter__()
        psD_cm = tc.tile_pool(name="psD", bufs=2, space="PSUM")
        psD = psD_cm.__enter__()
        h2T = bigD.tile([P, 8, S], f32r)
        xm_all = pDx.tile([P, 4, C], f32)
        for ch in range(2):
            pw = pD.tile([P, 8, 512], f32r, tag="pw_t")
            nc.sync.dma_start(
                pw[:], proj_w[:, ch * 512:(ch + 1) * 512].rearrange(
                    "(ko ki) m -> ki ko m", ki=P))
            for mt in range(4):
                ps = psD.tile([P, 512], f32, tag="proj_ps")
                for kt_i in range(8):
                    nc.tensor.matmul(ps[:], oT[:, kt_i, mt * P:(mt + 1) * P],
                                     pw[:, kt_i, :],
                                     start=(kt_i == 0), stop=(kt_i == 7))
                xqr = pD.tile([P, 512], f32, tag="xq_re")
                nc.sync.dma_start(xqr[:], x_q[(mt + 1) * P:(mt + 2) * P,
                                              ch * 512:(ch + 1) * 512])
                nc.vector.tensor_tensor(xm_all[:, mt, ch * 512:(ch + 1) * 512],
                                        ps[:], xqr[:], Alu.add)
        for mt in range(4):
            xm = xm_all[:, mt, :]
            nc.sync.dma_start(o_xmid[mt * P:(mt + 1) * P, :], xm)
            h2 = ln_tile(pD, xm)
            h2f = pD.tile([P, C], f32r, tag="h2f")
            nc.vector.tensor_tensor(h2f[:], h2[:].bitcast(f32), l2w[:], Alu.mult)
            nc.vector.tensor_tensor(h2f[:], h2f[:].bitcast(f32), l2b[:], Alu.add)
            h2b = pD.tile([P, C], bf16, tag="h2b")
            nc.vector.tensor_copy(h2b[:], h2f[:].bitcast(f32))
            nc.sync.dma_start(cc_in[mt * P:(mt + 1) * P, 0:C], h2b[:])
            transpose_into(psD, h2f, h2T, mt)
        rwt = pD.tile([P, 8, E], f32r, tag="rw_t")
        nc.sync.dma_start(rwt[:], rw[:, :].rearrange("(ko ki) e -> ki ko e", ki=P))
        ps_l = psD.tile([E, S], f32, tag="lg_ps")
        for kt_i in range(8):
            nc.tensor.matmul(ps_l[:], rwt[:, kt_i, :], h2T[:, kt_i, :],
                             start=(kt_i == 0), stop=(kt_i == 7))
        lgT = pD.tile([P, S], f32, tag="lgT")
        nc.vector.memset(lgT[:], 0.0)
        nc.vector.tensor_copy(lgT[0:E, :], ps_l[:])
        for tt in range(4):
            pst = psD.tile([P, P], f32, tag="lgtr_ps")
            nc.tensor.transpose(pst[:], lgT[:, tt * P:(tt + 1) * P],
                                ident[:].bitcast(f32))
            lgt = pD.tile([P, E], f32, tag="lg_tm")
            nc.vector.tensor_copy(lgt[:], pst[:, 0:E])
            nc.sync.dma_start(cc_in[tt * P:(tt + 1) * P, C:C + 2 * E],
                              lgt[:].bitcast(bf16))
        psD_cm.__exit__(None, None, None)
        pD_cm.__exit__(None, None, None)
        pDx_cm.__exit__(None, None, None)
        bigD_cm.__exit__(None, None, None)

        # ============ PHASE E: AllGather + routing + dispatch ============
        nc.gpsimd.collective_compute(
            "AllGather", Alu.bypass,
            ins=[cc_in[:, :]], outs=[cc_out[:, :]],
            replica_groups=[list(range(8))],
        )

        bigF_cm = tc.tile_pool(name="bigF", bufs=1)
        bigF = bigF_cm.__enter__()
        bigE_cm = tc.tile_pool(name="bigE", bufs=1)
        bigE = bigE_cm.__enter__()
        pE_cm = tc.tile_pool(name="pE", bufs=2)
        pE = pE_cm.__enter__()
        psE_cm = tc.tile_pool(name="psE", bufs=2, space="PSUM")
        psE = psE_cm.__enter__()

        lw = pE.tile([16, FW, E], f32)
        nc.sync.dma_start(
            lw[:], cc_out[:, C:C + 2 * E].bitcast(f32).rearrange(
                "(f p) e -> p f e", p=16))
        lmax = pE.tile([16, FW, 1], f32)
        nc.vector.tensor_reduce(lmax[:], lw[:], axis=mybir.AxisListType.X, op=Alu.max)
        mask_e = pE.tile([16, FW, E], f32)
        nc.vector.tensor_tensor(mask_e[:], lw[:], lmax[:].to_broadcast((16, FW, E)),
                                Alu.is_ge)
        oh = pE.tile([16, E], f32)
        nc.sync.dma_start(oh[:], onehot16[:, :])
        moh = pE.tile([16, FW, E], f32)
        nc.vector.tensor_tensor(moh[:], mask_e[:],
                                oh[:, None, :].to_broadcast((16, FW, E)), Alu.mult)
        m_mine = pE.tile([16, FW, 1], f32)
        nc.vector.tensor_reduce(m_mine[:], moh[:], axis=mybir.AxisListType.X, op=Alu.max)
        iota = pE.tile([16, FW], i32)
        nc.gpsimd.iota(iota[:], pattern=[[16, FW]], base=0, channel_multiplier=1)
        iota_f = pE.tile([16, FW], f32)
        nc.vector.tensor_copy(iota_f[:], iota[:])
        bnd = pE.tile([16, 2], f32)
        nc.sync.dma_start(bnd[:], bounds16[:, :])
        m1 = pE.tile([16, FW], f32)
        nc.vector.tensor_scalar(m1[:], iota_f[:], scalar1=bnd[:, 0:1], scalar2=None,
                                op0=Alu.is_ge)
        m2 = pE.tile([16, FW], f32)
        nc.vector.tensor_scalar(m2[:], iota_f[:], scalar1=bnd[:, 1:2], scalar2=None,
                                op0=Alu.is_lt)
        nc.vector.tensor_tensor(m1[:], m1[:], m2[:], Alu.mult)
        nc.vector.tensor_tensor(m1[:], m1[:], m_mine[:, :, 0], Alu.mult)
        ip1 = pE.tile([16, FW], f32)
        nc.vector.tensor_scalar(ip1[:], iota_f[:], scalar1=1.0, scalar2=None,
                                op0=Alu.add)
        cand = pE.tile([16, FW], f32)
        nc.vector.tensor_tensor(cand[:], ip1[:], m1[:], Alu.mult)
        nc.vector.tensor_scalar(cand[:], cand[:], scalar1=1.0, scalar2=None,
                                op0=Alu.subtract)
        ids = pE.tile([16, FW], f32)
        cnt = pE.tile([1, 1], u32)
        nc.gpsimd.sparse_gather(ids[:, :], cand[:], num_found=cnt[:])
        cntf = pE.tile([1, 1], f32)
        nc.vector.tensor_copy(cntf[:], cnt[:])
        nc.sync.dma_start(o_cnt[:, :], cntf[:])
        cnt16 = pE.tile([16, 1], f32)
        nc.gpsimd.partition_broadcast(cnt16[:], cntf[:, :])
        slot = pE.tile([16, CAP2 // 16], i32)
        nc.gpsimd.iota(slot[:], pattern=[[16, CAP2 // 16]], base=0,
                       channel_multiplier=1)
        slot_f = pE.tile([16, CAP2 // 16], f32)
        nc.vector.tensor_copy(slot_f[:], slot[:])
        valid = pE.tile([16, CAP2 // 16], f32)
        nc.vector.tensor_scalar(valid[:], slot_f[:], scalar1=cnt16[:], scalar2=None,
                                op0=Alu.is_lt)
        ids2 = pE.tile([16, CAP2 // 16], f32)
        nc.vector.tensor_scalar(ids2[:], ids[:, 0:CAP2 // 16], scalar1=1.0,
                                scalar2=None, op0=Alu.add)
        nc.vector.tensor_tensor(ids2[:], ids2[:], valid[:], Alu.mult)
        nc.vector.tensor_scalar(ids2[:], ids2[:], scalar1=1.0, scalar2=None,
                                op0=Alu.subtract)
        nc.sync.dma_start(o_idx[:, :], ids2[:])
        idx16 = pE.tile([P, CAP2 // 16], i16)
        nc.vector.tensor_copy(idx16[0:16, :], ids2[:])
        nc.sync.dma_start(idx_dram[:, :], idx16[0:16, :])
        idx_bcast_ap = bass.AP(
            tensor=idx_dram, offset=0,
            ap=[[0, 8], [CAP2 // 16, 16], [1, CAP2 // 16]])
        nc.gpsimd.dma_start(idx16[:, :], idx_bcast_ap)
        hTe = bigE.tile([P, 8, CAP2], bf16)
        nc.gpsimd.dma_gather(out_ap=hTe[:], in_ap=cc_out[:, 0:C], idxs_ap=idx16[:],
                             num_idxs=CAP2, num_idxs_reg=CAP2, elem_size=C,
                             elem_step=CCW, transpose=True)
        psE_cm.__exit__(None, None, None)
        pE_cm.__exit__(None, None, None)

        # ============ PHASE F+G: expert MLP, stage-2 interleaved per chunk ====
        pF_cm = tc.tile_pool(name="pF", bufs=3)
        pF = pF_cm.__enter__()
        pew_cm = tc.tile_pool(name="pew", bufs=1)
        pew = pew_cm.__enter__()
        psF_cm = tc.tile_pool(name="psF", bufs=2, space="PSUM")
        psF = psF_cm.__enter__()
        uT = bigF.tile([P, Dff // P, CAP2], bf16)
        pe_w0 = pew.tile([P, Dff // P, 512], bf16, tag="pe_w0")
        nc.sync.dma_start(pe_w0[:], proj_e[:, 0:512].rearrange(
            "(ko ki) m -> ki ko m", ki=P))
        pe_w1 = pew.tile([P, Dff // P, 512], bf16, tag="pe_w1")
        nc.sync.dma_start(pe_w1[:], proj_e[:, 512:1024].rearrange(
            "(ko ki) m -> ki ko m", ki=P))
        MTOF = {0: range(0, 3), 384: range(3, 5)}
        for c0, cw in [(0, 384), (384, 256)]:
            for dg in range(Dff // P // 4):
                wg = pF.tile([P, 8, 4 * P], bf16, tag="wv_g")
                nc.sync.dma_start(wg[:], wv_e[:, dg * 4 * P:(dg + 1) * 4 * P].rearrange(
                    "(ko ki) m -> ki ko m", ki=P))
                wvv = pF.tile([P, 8, 4 * P], bf16, tag="wv_v")
                nc.sync.dma_start(
                    wvv[:], wv_e[:, Dff + dg * 4 * P:Dff + (dg + 1) * 4 * P].rearrange(
                        "(ko ki) m -> ki ko m", ki=P))
                for dj in range(4):
                    dfft = dg * 4 + dj
                    ps_g = psF.tile([P, cw], f32, tag="st1g")
                    ps_v = psF.tile([P, cw], f32, tag="st1v")
                    for kt_i in range(8):
                        nc.tensor.matmul(ps_g[:], wg[:, kt_i, dj * P:(dj + 1) * P],
                                         hTe[:, kt_i, c0:c0 + cw],
                                         start=(kt_i == 0), stop=(kt_i == 7))
                    for kt_i in range(8):
                        nc.tensor.matmul(ps_v[:], wvv[:, kt_i, dj * P:(dj + 1) * P],
                                         hTe[:, kt_i, c0:c0 + cw],
                                         start=(kt_i == 0), stop=(kt_i == 7))
                    sg = pF.tile([P, cw], f32, tag="silu")
                    nc.scalar.activation(sg[:], ps_g[:], Act.Silu)
                    nc.vector.tensor_tensor(uT[:, dfft, c0:c0 + cw], sg[:], ps_v[:],
                                            Alu.mult)
            # stage-2 for the token tiles fully covered by this chunk
            for mt in MTOF[c0]:
                for chw, pe_w in ((0, pe_w0), (1, pe_w1)):
                    ps = psF.tile([P, 512], f32, tag="st2_ps")
                    for kt_i in range(Dff // P):
                        nc.tensor.matmul(ps[:], uT[:, kt_i, mt * P:(mt + 1) * P],
                                         pe_w[:, kt_i, :],
                                         start=(kt_i == 0), stop=(kt_i == Dff // P - 1))
                    yo = pF.tile([P, 512], f32, tag="y_out")
                    nc.vector.tensor_copy(yo[:], ps[:])
                    nc.sync.dma_start(
                        o_moe[mt * P:(mt + 1) * P, chw * 512:(chw + 1) * 512], yo[:])
        psF_cm.__exit__(None, None, None)
        pew_cm.__exit__(None, None, None)
        pF_cm.__exit__(None, None, None)
        bigE_cm.__exit__(None, None, None)
        bigF_cm.__exit__(None, None, None)
        consts_cm.__exit__(None, None, None)

    nc.finalize()
    return nc


def _rope_tables_np():
    inv_freq = 1.0 / (10000.0 ** (np.arange(0, D, 2, dtype=np.float32) / np.float32(D)))
    t = np.arange(T, dtype=np.float32)
    freqs = np.outer(t, inv_freq).astype(np.float32)   # [T, D/2]
    emb = np.concatenate([freqs, freqs], axis=-1)      # [T, D]
    return np.cos(emb), np.sin(emb)


def _host_inputs(x, ln1_w, ln1_b, ln2_w, ln2_b, conv_w, attn_w, proj_w,
                 router_w, expert_wv, expert_proj):
    xf = np.ascontiguousarray(np.asarray(x, np.float32).reshape(TOK, C))
    cos, sin = _rope_tables_np()
    cos_cm0 = np.empty((P, T), np.float32)
    sin_cm0 = np.empty((P, T), np.float32)
    for r in range(P):
        d = r % 64
        cos_cm0[r] = cos[:, d]
        sgn = -1.0 if d < 32 else 1.0
        sin_cm0[r] = sgn * sin[:, d]

    def cmap(a):
        return np.ascontiguousarray(np.asarray(a, np.float32).reshape(8, P).T)

    w3 = np.ascontiguousarray(
        np.asarray(conv_w, np.float32)[:, 0, :].reshape(8, P, 3).transpose(1, 0, 2))

    common = {
        "attn_w": np.asarray(attn_w, np.float32),
        "proj_w": np.asarray(proj_w, np.float32),
        "conv_w3": w3,
        "ln1_w2": cmap(ln1_w), "ln1_b2": cmap(ln1_b),
        "ln2_w_rep": np.tile(np.asarray(ln2_w, np.float32)[None, :], (P, 1)),
        "ln2_b_rep": np.tile(np.asarray(ln2_b, np.float32)[None, :], (P, 1)),
        "rw": np.asarray(router_w, np.float32),
        "identity": np.eye(P, dtype=np.float32),
        "ones16": np.ones((P, 16), np.float32),
        "ones_row": np.ones((1, P), np.float32),
        "ones_big": np.ones((P, P), np.float32),
    }
    ins = []
    kk = np.arange(T).reshape(8, P)
    for c in range(8):
        b = c // 2
        q0 = (c % 2) * S
        bt = xf[b * T:(b + 1) * T]
        x_kv = np.zeros((1152, C), np.float32)
        x_kv[128:] = bt
        x_q = np.zeros((640, C), np.float32)
        x_q[128:] = bt[q0:q0 + S]
        if q0 >= 128:
            x_q[0:128] = bt[q0 - 128:q0]
        qq = q0 + np.arange(S)
        mk = (kk[:, :, None] <= qq[None, None, :]).astype(ml_dtypes.bfloat16)
        e_mine = c // 2
        lo = (c % 2) * (TOK // 2)
        ins.append(dict(common,
                        x_q=x_q, x_kv=x_kv, masks=mk,
                        cos_cm=np.concatenate([cos_cm0, cos_cm0[:, q0:q0 + S]], 1).astype(ml_dtypes.bfloat16),
                        sin_cm=np.concatenate([sin_cm0, sin_cm0[:, q0:q0 + S]], 1).astype(ml_dtypes.bfloat16),
                        wv_e=np.ascontiguousarray(np.asarray(expert_wv, np.float32)[e_mine]).astype(ml_dtypes.bfloat16),
                        proj_e=np.ascontiguousarray(np.asarray(expert_proj, np.float32)[e_mine]).astype(ml_dtypes.bfloat16),
                        onehot16=np.tile(np.eye(E, dtype=np.float32)[e_mine][None, :], (16, 1)),
                        bounds16=np.tile(np.array([[lo, lo + TOK // 2]], np.float32), (16, 1)),
                        ))
    return ins


def kernel(**inputs):
    if "nc" not in _CACHE:
        _CACHE["nc"] = _build_nc()
    nc = _CACHE["nc"]
    in_maps = _host_inputs(**inputs)
    res = run_bass_kernel_spmd(nc, in_maps, core_ids=list(range(8)))
    x_mid = np.concatenate([res.results[c]["o_xmid"] for c in range(8)], axis=0)
    out = x_mid.copy()
    for c in range(8):
        idsw = res.results[c]["o_idx"]
        ids = np.rint(idsw.T.reshape(-1)).astype(np.int64)  # slot j = [j%16, j//16]
        y = res.results[c]["o_moe"]
        sel = ids >= 0
        out[ids[sel]] += y[sel]
    return out.reshape(B, T, C).astype(np.float32)



# revision 12
# speedup vs baseline: 1.5580x; 1.5580x over previous
"""Trainium2 Bass kernel for nn_Block_60017872995079 (moe_routing).

Block: x = x + attn(LN1(x)) ; x = x + moe(LN2(x))
  attn: LN1 -> causal depthwise conv1d(k=3) -> qkv -> RoPE -> causal MHA -> proj
  moe: LN2 -> router argmax (4 experts) -> per-token expert GLU-MLP (SwiGLU)

Sharding over 8 NeuronCores (single SPMD program, per-core behavior driven
entirely by per-core input DATA so every core runs identical code):
  - Attention: token-parallel. Core c owns tokens [512c, 512c+512) (batch c//2).
    Host ships per-core x_q (own tokens + 128 ctx rows), x_kv (full batch,
    zero ctx rows) and per-core causal 0/1 masks.
  - MoE: expert-parallel. Core c owns expert c//2 and the tokens routed to it
    from global half c%2. Routing is computed on device: argmax via
    max/compare masks, stream compaction via sparse_gather, token dispatch via
    dma_gather (capacity 640 tokens/core, expected ~512, -1-padded tail).
  - LN2 output + router logits are AllGathered (one collective) so each core
    can gather any token's h2. MoE outputs return compact (640 rows + token
    ids); host does the final unpermute-add (the unshard step).

All matmuls run in float32r (fp32 bit layout, reduced-precision PE mode, full
speed at free-dim>=256; ~1.4e-4 rel err per matmul).
"""

import sys
import numpy as np
import ml_dtypes

sys.path.insert(0, "/opt/trn_rl_repo")

import concourse.bass as bass  # noqa: E402,F401
import concourse.tile as tile  # noqa: E402
import concourse.mybir as mybir  # noqa: E402
from concourse import bacc  # noqa: E402
from concourse.bass_utils import run_bass_kernel_spmd  # noqa: E402

f32 = mybir.dt.float32
f32r = mybir.dt.float32r
bf16 = mybir.dt.bfloat16
fp8 = mybir.dt.float8e4
i16 = mybir.dt.int16
i32 = mybir.dt.int32
u32 = mybir.dt.uint32
Alu = mybir.AluOpType
Act = mybir.ActivationFunctionType

P = 128
B, T, C, NH, D, E, Dff = 4, 1024, 1024, 16, 4, 4, 4096
D = 64
TOK = B * T            # 4096
S = TOK // 8           # 512 tokens per core
K = 192                # AllToAll capacity per (src core, expert)
KC = K // 16           # idx cols per bucket (12)
CAPX = 4 * K           # expert slots per dst core (768)
WX = 8 * K             # payload row bytes per block (1536, %256==0)
EPS = 1e-5
FSCL = 64.0            # fp8 weight scale (wv gate half, proj)
VSCL = 32.0            # fp8 weight scale (wv value half; keeps uT under e4m3 max)

_CACHE = {}


def _build_nc():
    nc = bacc.Bacc(None)

    x_q = nc.dram_tensor("x_q", [640, C], f32, kind="ExternalInput")
    x_kv = nc.dram_tensor("x_kv", [1152, C], f32, kind="ExternalInput")
    attn_w = nc.dram_tensor("attn_w", [C, 3 * C], f32r, kind="ExternalInput")
    proj_w = nc.dram_tensor("proj_w", [C, C], f32r, kind="ExternalInput")
    conv_w3 = nc.dram_tensor("conv_w3", [P, 8, 3], f32, kind="ExternalInput")
    ln1_w2 = nc.dram_tensor("ln1_w2", [P, 8], f32, kind="ExternalInput")
    ln1_b2 = nc.dram_tensor("ln1_b2", [P, 8], f32, kind="ExternalInput")
    ln2_w_rep = nc.dram_tensor("ln2_w_rep", [P, C], f32, kind="ExternalInput")
    ln2_b_rep = nc.dram_tensor("ln2_b_rep", [P, C], f32, kind="ExternalInput")
    cos_cm = nc.dram_tensor("cos_cm", [P, T + S], bf16, kind="ExternalInput")
    sin_cm = nc.dram_tensor("sin_cm", [P, T + S], bf16, kind="ExternalInput")
    rw = nc.dram_tensor("rw", [C, E], f32r, kind="ExternalInput")
    masks = nc.dram_tensor("masks", [8, P, S], bf16, kind="ExternalInput")
    wv_dr = nc.dram_tensor("wv_dr", [P, 4, 2, 2 * Dff], fp8, kind="ExternalInput")
    pe_dr = nc.dram_tensor("pe_dr", [P, Dff // P, C], fp8, kind="ExternalInput")
    t0p1_16 = nc.dram_tensor("t0p1_16", [16, 1], f32, kind="ExternalInput")
    idxd16 = nc.dram_tensor("idxd16", [16, 32], i16, kind="ExternalInput")
    identity = nc.dram_tensor("identity", [P, P], f32r, kind="ExternalInput")
    ones16 = nc.dram_tensor("ones16", [P, 16], f32r, kind="ExternalInput")
    ones_row = nc.dram_tensor("ones_row", [1, P], f32r, kind="ExternalInput")
    ones_big = nc.dram_tensor("ones_big", [P, P], f32r, kind="ExternalInput")

    o_xmid = nc.dram_tensor("o_xmid", [S, C], f32, kind="ExternalOutput")
    o_moe = nc.dram_tensor("o_moe", [CAPX, C], f32, kind="ExternalOutput")
    o_ids = nc.dram_tensor("o_ids", [16, 4 * KC], f32, kind="ExternalOutput")

    h2_dram = nc.dram_tensor("h2_dram", [S, C], fp8)
    mask_dram = nc.dram_tensor("mask_dram", [S, E], f32)
    idx_dram = nc.dram_tensor("idx_dram", [16, 4 * KC], i16)
    a2a_in = nc.dram_tensor("a2a_in", [8 * P, WX], fp8)
    a2a_out = nc.dram_tensor("a2a_out", [8 * P, WX], fp8)


    with tile.TileContext(nc) as tc, nc.allow_low_precision(reason="f32r pipeline"):
        consts_cm = tc.tile_pool(name="consts", bufs=1)
        consts = consts_cm.__enter__()
        ident = consts.tile([P, P], f32r)
        nc.sync.dma_start(ident[:], identity[:, :])
        one16 = consts.tile([P, 16], f32r)
        nc.sync.dma_start(one16[:], ones16[:, :])
        onr = consts.tile([1, P], f32r)
        nc.sync.dma_start(onr[:], ones_row[:, :])
        onesb = consts.tile([P, P], f32r)
        nc.sync.dma_start(onesb[:], ones_big[:, :])
        cw3 = consts.tile([P, 8, 3], f32)
        nc.sync.dma_start(cw3[:], conv_w3[:, :, :])
        l1w = consts.tile([P, 8], f32)
        nc.sync.dma_start(l1w[:], ln1_w2[:, :])
        l1b = consts.tile([P, 8], f32)
        nc.sync.dma_start(l1b[:], ln1_b2[:, :])
        l2w = consts.tile([P, C], f32)
        nc.sync.dma_start(l2w[:], ln2_w_rep[:, :])
        l2b = consts.tile([P, C], f32)
        nc.sync.dma_start(l2b[:], ln2_b_rep[:, :])
        epst = consts.tile([P, 1], f32)
        nc.vector.memset(epst[:], EPS)

        BN_F = nc.vector.BN_STATS_FMAX  # 512
        n_sub = C // BN_F

        def ln_tile(pool, x_sb):
            stats = pool.tile([P, n_sub, nc.vector.BN_STATS_DIM], f32, tag="ln_stats")
            for sgi in range(n_sub):
                nc.vector.bn_stats(stats[:, sgi, :],
                                   x_sb[:, sgi * BN_F:(sgi + 1) * BN_F])
            mv = pool.tile([P, nc.vector.BN_AGGR_DIM], f32, tag="ln_mv")
            nc.vector.bn_aggr(mv[:], stats[:])
            sd = pool.tile([P, 1], f32, tag="ln_sd")
            nc.scalar.activation(sd[:], mv[:, 1:2], Act.Sqrt, bias=epst[:], scale=1.0)
            nc.vector.reciprocal(sd[:], sd[:])
            xn = pool.tile([P, C], f32r, tag="ln_xn")
            nc.vector.tensor_scalar(xn[:], x_sb[:], scalar1=mv[:, 0:1], scalar2=sd[:],
                                    op0=Alu.subtract, op1=Alu.mult)
            return xn

        def transpose_into(psum_pool, src_tile, dst, ti):
            """PE-transpose one [P, C] f32r token-tile into dst[:, ct, ti*128..]."""
            for cg in range(2):
                pst = psum_pool.tile([P, 512], f32r, tag="tr_ps")
                for cj in range(4):
                    ct = cg * 4 + cj
                    nc.tensor.transpose(pst[:, cj * P:(cj + 1) * P],
                                        src_tile[:, ct * P:(ct + 1) * P], ident[:])
                nc.vector.tensor_copy(
                    dst[:, cg * 4:(cg + 1) * 4, ti * P:(ti + 1) * P],
                    pst[:].rearrange("p (a b) -> p a b", b=P))

        def rope_cm(pool, ps_in, out_sb, out_sl, tcol0, w):
            shuf = pool.tile([P, w], f32, tag="rp_sh")
            smask = [8, 9, 10, 11, 12, 13, 14, 15, 0, 1, 2, 3, 4, 5, 6, 7,
                     24, 25, 26, 27, 28, 29, 30, 31, 16, 17, 18, 19, 20, 21, 22, 23]
            nc.vector.stream_shuffle(shuf[:], ps_in[:], smask)
            t1 = pool.tile([P, w], f32, tag="rp_t1")
            nc.vector.tensor_tensor(t1[:], ps_in[:], cosc[:, tcol0:tcol0 + w], Alu.mult)
            nc.vector.tensor_tensor(shuf[:], shuf[:], sinc[:, tcol0:tcol0 + w], Alu.mult)
            nc.vector.tensor_tensor(out_sb[out_sl], t1[:], shuf[:], Alu.add)

        # pool stack (LIFO): consts > bigD > bigB > [phase pools]
        bigD_cm = tc.tile_pool(name="bigD", bufs=1)
        bigD = bigD_cm.__enter__()
        bigB_cm = tc.tile_pool(name="bigB", bufs=1)
        bigB = bigB_cm.__enter__()

        # ============ PHASE A: LN1 + transpose + conv ============
        bigA_cm = tc.tile_pool(name="bigA", bufs=1)
        bigA = bigA_cm.__enter__()
        pA_cm = tc.tile_pool(name="pA", bufs=3)
        pA = pA_cm.__enter__()
        psA_cm = tc.tile_pool(name="psA", bufs=2, space="PSUM")
        psA = psA_cm.__enter__()

        h1t_kv = bigA.tile([P, 8, 1152], f32r)
        h1t_q = bigA.tile([P, 8, 640], f32r)
        for i in range(9):
            xt = pA.tile([P, C], f32, tag="xkv_in")
            nc.sync.dma_start(xt[:], x_kv[i * P:(i + 1) * P, :])
            transpose_into(psA, ln_tile(pA, xt), h1t_kv, i)
        for i in range(5):
            xt = pA.tile([P, C], f32, tag="xkv_in")
            nc.sync.dma_start(xt[:], x_q[i * P:(i + 1) * P, :])
            transpose_into(psA, ln_tile(pA, xt), h1t_q, i)

        hct_kv = bigB.tile([P, 8, T], f32r)
        hct_q = bigB.tile([P, 8, S], f32r)
        for ct in range(8):
            nc.vector.tensor_scalar(h1t_kv[:, ct, :], h1t_kv[:, ct, :],
                                    scalar1=l1w[:, ct:ct + 1], scalar2=l1b[:, ct:ct + 1],
                                    op0=Alu.mult, op1=Alu.add)
            nc.vector.tensor_scalar(h1t_q[:, ct, :], h1t_q[:, ct, :],
                                    scalar1=l1w[:, ct:ct + 1], scalar2=l1b[:, ct:ct + 1],
                                    op0=Alu.mult, op1=Alu.add)
        for src, dst, n in ((h1t_kv, hct_kv, T), (h1t_q, hct_q, S)):
            for ct in range(8):
                nc.gpsimd.tensor_scalar(dst[:, ct, :], src[:, ct, 126:126 + n],
                                        scalar1=cw3[:, ct, 0:1], scalar2=None,
                                        op0=Alu.mult)
                nc.vector.scalar_tensor_tensor(dst[:, ct, :], src[:, ct, 127:127 + n],
                                               cw3[:, ct, 1:2], dst[:, ct, :],
                                               op0=Alu.mult, op1=Alu.add)
                nc.vector.scalar_tensor_tensor(dst[:, ct, :], src[:, ct, 128:128 + n],
                                               cw3[:, ct, 2:3], dst[:, ct, :],
                                               op0=Alu.mult, op1=Alu.add)
        psA_cm.__exit__(None, None, None)
        pA_cm.__exit__(None, None, None)
        bigA_cm.__exit__(None, None, None)

        # ============ PHASE B+C: qkv + rope + attention, fused per head-pair ==
        pV_cm = tc.tile_pool(name="pV", bufs=1)
        pV = pV_cm.__enter__()
        pB_cm = tc.tile_pool(name="pB", bufs=2)
        pB = pB_cm.__enter__()
        pBs_cm = tc.tile_pool(name="pBs", bufs=2)
        pBs = pBs_cm.__enter__()
        psB_cm = tc.tile_pool(name="psB", bufs=2, space="PSUM")
        psB = psB_cm.__enter__()

        cosc = bigD.tile([P, T + S], bf16)
        nc.sync.dma_start(cosc[:], cos_cm[:, :])
        sinc = bigD.tile([P, T + S], bf16)
        nc.sync.dma_start(sinc[:], sin_cm[:, :])
        msk = bigD.tile([P, 8, S], bf16)
        nc.sync.dma_start(msk[:], masks[:, :, :].rearrange("k p q -> p k q"))
        oT = bigD.tile([P, 8, S], f32r)
        vt_all = pV.tile([P, 8, 16, 65], f32r)
        for ch in range(2):
            wtv = pV.tile([P, 8, 512], f32r, tag="aw_v5")
            nc.sync.dma_start(
                wtv[:], attn_w[:, 2 * C + ch * 512:2 * C + (ch + 1) * 512].rearrange(
                    "(ko ki) m -> ki ko m", ki=P))
            for tt in range(8):
                ps = psB.tile([P, 512], f32, tag="qkv_ps")
                for kt_i in range(8):
                    nc.tensor.matmul(ps[:], hct_kv[:, kt_i, tt * P:(tt + 1) * P],
                                     wtv[:, kt_i, :],
                                     start=(kt_i == 0), stop=(kt_i == 7))
                nc.scalar.copy(vt_all[:, tt, ch * 8:(ch + 1) * 8, 0:64],
                               ps[:].rearrange("p (h d) -> p h d", d=64))
        nc.sync.dma_start(
            vt_all[:, :, :, 64:65],
            onesb[:, 0:128].rearrange("p (a b) -> p a b", b=16)[:, :, :, None])

        for ct in range(8):
            # --- K channel-major for this chan tile (2 heads) ---
            kTc = pB.tile([P, T], f32r, tag="kTc")
            wt = pBs.tile([P, 8, P], f32r, tag="aw")
            nc.sync.dma_start(wt[:], attn_w[:, C + ct * P:C + (ct + 1) * P].rearrange(
                "(ko ki) m -> ki ko m", ki=P))
            for ch in range(2):
                ps = psB.tile([P, 512], f32, tag="qkv_ps")
                for kt_i in range(8):
                    nc.tensor.matmul(ps[:], wt[:, kt_i, :],
                                     hct_kv[:, kt_i, ch * 512:(ch + 1) * 512],
                                     start=(kt_i == 0), stop=(kt_i == 7))
                rope_cm(pBs, ps, kTc, (slice(None), slice(ch * 512, (ch + 1) * 512)),
                        ch * 512, 512)
            # --- Q ---
            qTc = pB.tile([P, S], f32r, tag="qTc")
            wt = pBs.tile([P, 8, P], f32r, tag="aw")
            nc.sync.dma_start(wt[:], attn_w[:, ct * P:(ct + 1) * P].rearrange(
                "(ko ki) m -> ki ko m", ki=P))
            ps = psB.tile([P, 512], f32, tag="qkv_ps")
            for kt_i in range(8):
                nc.tensor.matmul(ps[:], wt[:, kt_i, :], hct_q[:, kt_i, :],
                                 start=(kt_i == 0), stop=(kt_i == 7))
            rope_cm(pBs, ps, qTc, (slice(None), slice(0, S)), T, S)
            # --- attention for heads 2ct, 2ct+1 ---
            for hh in range(2):
                po = 64 * hh
                ps_o = psB.tile([65, S], f32, tag="pv_ps")
                for kt_i in range(8):
                    ps_s = psB.tile([P, S], f32, tag="s_ps")
                    nc.tensor.matmul(ps_s[:],
                                     kTc[po:po + 64, kt_i * P:(kt_i + 1) * P],
                                     qTc[po:po + 64, :], start=True, stop=True)
                    pt = pBs.tile([P, S], f32, tag="exp_t")
                    nc.scalar.activation(pt[:], ps_s[:], Act.Exp, scale=0.125)
                    ptm = pBs.tile([P, S], f32r, tag="ptm_t")
                    meng = nc.vector if kt_i % 2 == 0 else nc.gpsimd
                    meng.tensor_tensor(ptm[:], pt[:], msk[:, kt_i, :], Alu.mult)
                    nc.tensor.matmul(ps_o[:], vt_all[:, kt_i, 2 * ct + hh, :], ptm[:],
                                     start=(kt_i == 0), stop=(kt_i == 7))
                rc = pBs.tile([1, S], f32r, tag="rc_t")
                nc.vector.reciprocal(rc[:], ps_o[64:65, :])
                ps_b = psB.tile([64, S], f32, tag="bc_ps")
                nc.tensor.matmul(ps_b[:], onr[:, 0:64], rc[:], start=True, stop=True)
                bc = pBs.tile([64, S], f32, tag="bc_t")
                nc.scalar.copy(bc[:], ps_b[:])
                nc.vector.tensor_tensor(oT[po:po + 64, ct, :], ps_o[0:64, :], bc[:],
                                        Alu.mult)
        psB_cm.__exit__(None, None, None)
        pBs_cm.__exit__(None, None, None)
        pB_cm.__exit__(None, None, None)
        pV_cm.__exit__(None, None, None)
        bigB_cm.__exit__(None, None, None)

        # ============ PHASE D: proj + residual + LN2 + logits ============
        pDx_cm = tc.tile_pool(name="pDx", bufs=1)
        pDx = pDx_cm.__enter__()
        pD_cm = tc.tile_pool(name="pD", bufs=3)
        pD = pD_cm.__enter__()
        psD_cm = tc.tile_pool(name="psD", bufs=2, space="PSUM")
        psD = psD_cm.__enter__()
        h2T = bigD.tile([P, 8, S], f32r)
        xm_all = pDx.tile([P, 4, C], f32)
        for ch in range(2):
            pw = pD.tile([P, 8, 512], f32r, tag="pw_t")
            nc.sync.dma_start(
                pw[:], proj_w[:, ch * 512:(ch + 1) * 512].rearrange(
                    "(ko ki) m -> ki ko m", ki=P))
            for mt in range(4):
                ps = psD.tile([P, 512], f32, tag="proj_ps")
                for kt_i in range(8):
                    nc.tensor.matmul(ps[:], oT[:, kt_i, mt * P:(mt + 1) * P],
                                     pw[:, kt_i, :],
                                     start=(kt_i == 0), stop=(kt_i == 7))
                xqr = pD.tile([P, 512], f32, tag="xq_re")
                nc.sync.dma_start(xqr[:], x_q[(mt + 1) * P:(mt + 2) * P,
                                              ch * 512:(ch + 1) * 512])
                nc.vector.tensor_tensor(xm_all[:, mt, ch * 512:(ch + 1) * 512],
                                        ps[:], xqr[:], Alu.add)
        for mt in range(4):
            xm = xm_all[:, mt, :]
            nc.sync.dma_start(o_xmid[mt * P:(mt + 1) * P, :], xm)
            h2 = ln_tile(pD, xm)
            h2f = pD.tile([P, C], f32r, tag="h2f")
            nc.vector.tensor_tensor(h2f[:], h2[:].bitcast(f32), l2w[:], Alu.mult)
            nc.vector.tensor_tensor(h2f[:], h2f[:].bitcast(f32), l2b[:], Alu.add)
            h2q = pD.tile([P, C], fp8, tag="h2q")
            nc.vector.tensor_copy(h2q[:], h2f[:].bitcast(f32))
            nc.sync.dma_start(h2_dram[mt * P:(mt + 1) * P, :], h2q[:])
            transpose_into(psD, h2f, h2T, mt)
        rwt = pD.tile([P, 8, E], f32r, tag="rw_t")
        nc.sync.dma_start(rwt[:], rw[:, :].rearrange("(ko ki) e -> ki ko e", ki=P))
        ps_l = psD.tile([E, S], f32, tag="lg_ps")
        for kt_i in range(8):
            nc.tensor.matmul(ps_l[:], rwt[:, kt_i, :], h2T[:, kt_i, :],
                             start=(kt_i == 0), stop=(kt_i == 7))
        lgT = pD.tile([P, S], f32, tag="lgT")
        nc.vector.memset(lgT[:], 0.0)
        nc.vector.tensor_copy(lgT[0:E, :], ps_l[:])
        for tt in range(4):
            pst = psD.tile([P, P], f32, tag="lgtr_ps")
            nc.tensor.transpose(pst[:], lgT[:, tt * P:(tt + 1) * P],
                                ident[:].bitcast(f32))
            lmax = pD.tile([P, 1], f32, tag="lg_mx")
            nc.vector.tensor_reduce(lmax[:], pst[:, 0:E],
                                    axis=mybir.AxisListType.X, op=Alu.max)
            emask = pD.tile([P, E], f32, tag="lg_mk")
            nc.vector.tensor_tensor(emask[:], pst[:, 0:E],
                                    lmax[:].to_broadcast((P, E)), Alu.is_ge)
            nc.sync.dma_start(mask_dram[tt * P:(tt + 1) * P, :], emask[:])
        psD_cm.__exit__(None, None, None)
        pD_cm.__exit__(None, None, None)
        pDx_cm.__exit__(None, None, None)
        bigD_cm.__exit__(None, None, None)

        # ============ PHASE E: local routing + payload + AllToAll dispatch ====
        bigF_cm = tc.tile_pool(name="bigF", bufs=1)
        bigF = bigF_cm.__enter__()
        bigE_cm = tc.tile_pool(name="bigE", bufs=1)
        bigE = bigE_cm.__enter__()
        pE_cm = tc.tile_pool(name="pE", bufs=2)
        pE = pE_cm.__enter__()

        FW2 = S // 16  # 32 wrapped cols over own 512 tokens
        maskw = pE.tile([16, FW2, E], f32)
        nc.sync.dma_start(
            maskw[:], mask_dram[:, :].rearrange("(f p) e -> p f e", p=16))
        iota = pE.tile([16, FW2], i32)
        nc.gpsimd.iota(iota[:], pattern=[[16, FW2]], base=0, channel_multiplier=1)
        iota_f = pE.tile([16, FW2], f32)
        nc.vector.tensor_copy(iota_f[:], iota[:])
        t0p1 = pE.tile([16, 1], f32)
        nc.sync.dma_start(t0p1[:], t0p1_16[:, :])
        idx_all = pE.tile([16, 4 * KC], i16)
        gid_all = pE.tile([16, 4 * KC], f32)
        for e in range(E):
            cand = pE.tile([16, FW2], f32, tag="cand")
            nc.vector.tensor_scalar(cand[:], iota_f[:], scalar1=1.0, scalar2=None,
                                    op0=Alu.add)
            nc.vector.tensor_tensor(cand[:], cand[:], maskw[:, :, e], Alu.mult)
            nc.vector.tensor_scalar(cand[:], cand[:], scalar1=1.0, scalar2=None,
                                    op0=Alu.subtract)
            ids = pE.tile([16, FW2], f32, tag="ids")
            cnt = pE.tile([1, 1], u32, tag="cnt")
            nc.gpsimd.sparse_gather(ids[:, :], cand[:], num_found=cnt[:])
            cntf = pE.tile([1, 1], f32, tag="cntf")
            nc.vector.tensor_copy(cntf[:], cnt[:])
            cnt16 = pE.tile([16, 1], f32, tag="cnt16")
            nc.gpsimd.partition_broadcast(cnt16[:], cntf[:, :])
            valid = pE.tile([16, KC], f32, tag="valid")
            nc.vector.tensor_scalar(valid[:], iota_f[:, 0:KC], scalar1=cnt16[:],
                                    scalar2=None, op0=Alu.is_lt)
            ids2 = pE.tile([16, KC], f32, tag="ids2")
            nc.vector.tensor_scalar(ids2[:], ids[:, 0:KC], scalar1=1.0,
                                    scalar2=None, op0=Alu.add)
            nc.vector.tensor_tensor(ids2[:], ids2[:], valid[:], Alu.mult)
            nc.vector.tensor_scalar(ids2[:], ids2[:], scalar1=1.0, scalar2=None,
                                    op0=Alu.subtract)
            nc.vector.tensor_copy(idx_all[:, e * KC:(e + 1) * KC], ids2[:])
            gids = pE.tile([16, KC], f32, tag="gids")
            nc.vector.tensor_scalar(gids[:], ids[:, 0:KC], scalar1=t0p1[:],
                                    scalar2=None, op0=Alu.add)
            nc.vector.tensor_tensor(gids[:], gids[:], valid[:], Alu.mult)
            nc.vector.tensor_scalar(
                gid_all[:, e * KC:(e + 1) * KC], gids[:], scalar1=1.0,
                scalar2=None, op0=Alu.subtract)
        nc.sync.dma_start(o_ids[:, :], gid_all[:])
        nc.sync.dma_start(idx_dram[:, :], idx_all[:])
        idx_bc = pE.tile([P, 4 * KC], i16)
        idx_bcast_ap = bass.AP(
            tensor=idx_dram, offset=0,
            ap=[[0, 8], [4 * KC, 16], [1, 4 * KC]])
        nc.gpsimd.dma_start(idx_bc[:, :], idx_bcast_ap)
        # gather own h2 rows (fp8, channel-major with 16-bit interleave)
        ghT = bigE.tile([P, 4, CAPX, 2], fp8)
        gh_view = ghT[:].rearrange("p k t c -> p (k t c)").rearrange(
            "p (a b) -> p a b", b=CAPX)
        nc.gpsimd.dma_gather(out_ap=gh_view, in_ap=h2_dram[:, :],
                             idxs_ap=idx_bc[:], num_idxs=CAPX, num_idxs_reg=CAPX,
                             elem_size=C, elem_step=C, transpose=True)
        for e in range(E):
            for blk in (e, 4 + e):
                nc.sync.dma_start(
                    a2a_in[blk * P:(blk + 1) * P, :].rearrange(
                        "r (k t c) -> r k t c", k=4, c=2),
                    ghT[:, :, e * K:(e + 1) * K, :])
        nc.gpsimd.collective_compute(
            "AllToAll", Alu.bypass,
            ins=[a2a_in[:, :]], outs=[a2a_out[:, :]],
            replica_groups=[list(range(8))],
        )
        # pull my 4 source blocks (static per-core row indices from host data)
        idxd = pE.tile([P, 32], i16)
        idxd_bcast = bass.AP(
            tensor=idxd16, offset=0,
            ap=[[0, 8], [32, 16], [1, 32]])
        nc.gpsimd.dma_start(idxd[:, :], idxd_bcast)
        hTe = bigE.tile([P, 4, 4, K, 2], fp8)
        hTe_view = hTe[:].rearrange("p s k t c -> p (s k t c)").rearrange(
            "p (a b) -> p a b", b=WX)
        nc.gpsimd.dma_gather(out_ap=hTe_view, in_ap=a2a_out[:, :],
                             idxs_ap=idxd[:], num_idxs=512, num_idxs_reg=512,
                             elem_size=WX, elem_step=WX, transpose=False)
        pE_cm.__exit__(None, None, None)

        # ============ PHASE F+G: expert MLP, fp8 DoubleRow both stages ========
        pF_cm = tc.tile_pool(name="pF", bufs=3)
        pF = pF_cm.__enter__()
        pew_cm = tc.tile_pool(name="pew", bufs=1)
        pew = pew_cm.__enter__()
        psF_cm = tc.tile_pool(name="psF", bufs=2, space="PSUM")
        psF = psF_cm.__enter__()
        DR = mybir.MatmulPerfMode.DoubleRow
        uT = bigF.tile([P, Dff // P, CAPX], fp8)
        pe_w0 = pew.tile([P, Dff // P, 512], fp8, tag="pe_w0")
        nc.sync.dma_start(pe_w0[:], pe_dr[:, :, 0:512])
        pe_w1 = pew.tile([P, Dff // P, 512], fp8, tag="pe_w1")
        nc.sync.dma_start(pe_w1[:], pe_dr[:, :, 512:1024])
        for dg in range(Dff // P // 4):
            wg = pF.tile([P, 4, 2, 4 * P], fp8, tag="wv_g")
            nc.sync.dma_start(wg[:], wv_dr[:, :, :, dg * 4 * P:(dg + 1) * 4 * P])
            wvv = pF.tile([P, 4, 2, 4 * P], fp8, tag="wv_v")
            nc.sync.dma_start(
                wvv[:], wv_dr[:, :, :, Dff + dg * 4 * P:Dff + (dg + 1) * 4 * P])
            for dj in range(4):
                dfft = dg * 4 + dj
                for h in range(2):
                    ps_g = psF.tile([P, 2 * K], f32, tag="st1g")
                    ps_v = psF.tile([P, 2 * K], f32, tag="st1v")
                    for pst, wt in ((ps_g, wg), (ps_v, wvv)):
                        for s2 in range(2):
                            s = 2 * h + s2
                            for k in range(4):
                                nc.tensor.matmul(
                                    pst[:, s2 * K:(s2 + 1) * K],
                                    wt[:, k, :, dj * P:(dj + 1) * P],
                                    hTe[:, s, k, :, :].rearrange("p t c -> p c t"),
                                    start=(k == 0), stop=(k == 3),
                                    perf_mode=DR)
                    sg = pF.tile([P, 2 * K], f32, tag="silu")
                    nc.scalar.activation(sg[:], ps_g[:], Act.Silu, scale=1.0 / FSCL)
                    nc.vector.tensor_tensor(uT[:, dfft, h * 2 * K:(h + 1) * 2 * K],
                                            sg[:], ps_v[:], Alu.mult)
        for mt in range(CAPX // P + (1 if CAPX % P else 0)):
            mp = min(P, CAPX - mt * P)
            for chw, pe_w in ((0, pe_w0), (1, pe_w1)):
                ps = psF.tile([P, 512], f32, tag="st2_ps")
                for k in range(Dff // P // 2):
                    nc.tensor.matmul(ps[0:mp, :],
                                     uT[:, 2 * k:2 * k + 2, mt * P:mt * P + mp],
                                     pe_w[:, 2 * k:2 * k + 2, :],
                                     start=(k == 0), stop=(k == Dff // P // 2 - 1),
                                     perf_mode=DR)
                yo = pF.tile([P, 512], f32, tag="y_out")
                nc.scalar.activation(yo[0:mp, :], ps[0:mp, :], Act.Copy,
                                     scale=1.0 / (VSCL * FSCL))
                nc.sync.dma_start(
                    o_moe[mt * P:mt * P + mp, chw * 512:(chw + 1) * 512],
                    yo[0:mp, :])
        psF_cm.__exit__(None, None, None)
        pew_cm.__exit__(None, None, None)
        pF_cm.__exit__(None, None, None)
        bigE_cm.__exit__(None, None, None)
        bigF_cm.__exit__(None, None, None)
        consts_cm.__exit__(None, None, None)

    nc.finalize()
    return nc


def _rope_tables_np():
    inv_freq = 1.0 / (10000.0 ** (np.arange(0, D, 2, dtype=np.float32) / np.float32(D)))
    t = np.arange(T, dtype=np.float32)
    freqs = np.outer(t, inv_freq).astype(np.float32)   # [T, D/2]
    emb = np.concatenate([freqs, freqs], axis=-1)      # [T, D]
    return np.cos(emb), np.sin(emb)


def _host_inputs(x, ln1_w, ln1_b, ln2_w, ln2_b, conv_w, attn_w, proj_w,
                 router_w, expert_wv, expert_proj):
    xf = np.ascontiguousarray(np.asarray(x, np.float32).reshape(TOK, C))
    cos, sin = _rope_tables_np()
    cos_cm0 = np.empty((P, T), np.float32)
    sin_cm0 = np.empty((P, T), np.float32)
    for r in range(P):
        d = r % 64
        cos_cm0[r] = cos[:, d]
        sgn = -1.0 if d < 32 else 1.0
        sin_cm0[r] = sgn * sin[:, d]

    def cmap(a):
        return np.ascontiguousarray(np.asarray(a, np.float32).reshape(8, P).T)

    w3 = np.ascontiguousarray(
        np.asarray(conv_w, np.float32)[:, 0, :].reshape(8, P, 3).transpose(1, 0, 2))

    common = {
        "attn_w": np.asarray(attn_w, np.float32),
        "proj_w": np.asarray(proj_w, np.float32),
        "conv_w3": w3,
        "ln1_w2": cmap(ln1_w), "ln1_b2": cmap(ln1_b),
        "ln2_w_rep": np.tile(np.asarray(ln2_w, np.float32)[None, :], (P, 1)),
        "ln2_b_rep": np.tile(np.asarray(ln2_b, np.float32)[None, :], (P, 1)),
        "rw": np.asarray(router_w, np.float32),
        "identity": np.eye(P, dtype=np.float32),
        "ones16": np.ones((P, 16), np.float32),
        "ones_row": np.ones((1, P), np.float32),
        "ones_big": np.ones((P, P), np.float32),
    }
    # fp8 expert weights, one expert per core pair-of-groups: core c -> expert c%4.
    # wv_dr[p, k, c2, m] = wv[256k + 2p + c2, m] * scale (gate x64 cols [0:Dff],
    # value x32 cols [Dff:2Dff]) -- the (2p + c2) interleave matches the 16-bit
    # granularity of the transposed fp8 dma_gather.
    wv_f = np.asarray(expert_wv, np.float32)
    pe_f = np.asarray(expert_proj, np.float32)
    wv_dr_all, pe_dr_all = [], []
    for e in range(E):
        wvs = wv_f[e].copy()
        wvs[:, :Dff] *= FSCL
        wvs[:, Dff:] *= VSCL
        wv_dr_all.append(np.ascontiguousarray(
            wvs.reshape(4, P, 2, 2 * Dff).transpose(1, 0, 2, 3)
        ).astype(ml_dtypes.float8_e4m3))
        pe_dr_all.append(np.ascontiguousarray(
            (pe_f[e] * FSCL).reshape(Dff // P, P, C).transpose(1, 0, 2)
        ).astype(ml_dtypes.float8_e4m3))

    ins = []
    kk = np.arange(T).reshape(8, P)
    jw = np.arange(512).reshape(32, 16).T  # wrapped [16, 32]: [j%16, j//16] = j
    for c in range(8):
        b = c // 2
        q0 = (c % 2) * S
        bt = xf[b * T:(b + 1) * T]
        x_kv = np.zeros((1152, C), np.float32)
        x_kv[128:] = bt
        x_q = np.zeros((640, C), np.float32)
        x_q[128:] = bt[q0:q0 + S]
        if q0 >= 128:
            x_q[0:128] = bt[q0 - 128:q0]
        qq = q0 + np.arange(S)
        mk = (kk[:, :, None] <= qq[None, None, :]).astype(ml_dtypes.bfloat16)
        e_mine = c % 4
        g = c // 4
        ins.append(dict(common,
                        x_q=x_q, x_kv=x_kv, masks=mk,
                        cos_cm=np.concatenate([cos_cm0, cos_cm0[:, q0:q0 + S]], 1).astype(ml_dtypes.bfloat16),
                        sin_cm=np.concatenate([sin_cm0, sin_cm0[:, q0:q0 + S]], 1).astype(ml_dtypes.bfloat16),
                        wv_dr=wv_dr_all[e_mine],
                        pe_dr=pe_dr_all[e_mine],
                        t0p1_16=np.full((16, 1), c * S + 1, np.float32),
                        idxd16=np.ascontiguousarray(512 * g + jw).astype(np.int16),
                        ))
    return ins


def kernel(**inputs):
    if "nc" not in _CACHE:
        _CACHE["nc"] = _build_nc()
    nc = _CACHE["nc"]
    in_maps = _host_inputs(**inputs)
    res = run_bass_kernel_spmd(nc, in_maps, core_ids=list(range(8)))
    x_mid = np.concatenate([res.results[c]["o_xmid"] for c in range(8)], axis=0)
    out = x_mid.copy()
    for d in range(8):
        g, e = d // 4, d % 4
        y = res.results[d]["o_moe"]          # [CAPX, C], slot tau = s*K + t
        for s in range(4):
            src = 4 * g + s
            gcols = res.results[src]["o_ids"][:, e * KC:(e + 1) * KC]  # [16, KC]
            ids = np.rint(np.nan_to_num(gcols.T.reshape(-1), nan=-1.0)
                          ).astype(np.int64)  # slot t = [t%16, t//16]
            sel = (ids >= 0) & (ids < TOK)
            out[ids[sel]] += y[s * K:(s + 1) * K][sel]
    return out.reshape(B, T, C).astype(np.float32)

